# revision 12
# baseline (speedup 1.0000x reference)
"""Trainium2 Bass kernel for nn_AimComms (vq_codebook).

Data-parallel over 8 NeuronCores: tokens (B*T*N = 16384) sharded 2048/core.

Per-core layout (S1): tokens on partitions for the softmax pipeline,
features on partitions for the cond/weight matmuls.

Key tricks:
- All matmuls bf16 (fp32 PE matmul is 4x slower); bias added exactly via a
  K=2 matmul against [b_hi; b_lo] (bf16 hi/lo split), codebook likewise.
- Codeword lookup = one-hot (device-built via DVE is_equal on fp16
  replicated indices) x codebook matmul, col-packed M=32 (tile_position).
- Cumulative-hard: PSUM accumulates h0, h0+h1, h0+h1+h2 across levels in one
  accumulation group (W2 is host-transformed so cond uses cumulative sums);
  the final PSUM state is comm_output directly.
- Softmax Z for free via ACT Exp accum_out; entropy dot-product sum(E*s) and
  the log-prob gather (sum(E * (iota==idx))) via fused scalar_tensor_tensor
  with accum_out on DVE.
- log p = s_idx - logZ computed as ln(E_idx) - ln(Z).
"""

import os
from contextlib import ExitStack

import numpy as np
import ml_dtypes

import concourse.bass as bass
import concourse.tile as tile
import concourse.mybir as mybir
from concourse import bacc, bass_utils
from concourse._compat import with_exitstack

BF16 = mybir.dt.bfloat16
F16 = mybir.dt.float16
F32 = mybir.dt.float32
AF = mybir.ActivationFunctionType
OP = mybir.AluOpType

P = 128
NCORES = 8
B, T, N = 32, 64, 8
NC_, L, C, V, H = 4, 3, 32, 512, 512
S = B * T * N            # 16384 tokens total
SC = S // NCORES         # 2048 tokens per core
NTILES = SC // P         # 16 token tiles per core
FOUT = NC_ * V           # 2048 logits per level
FAN = [H + l * NC_ * C for l in range(L)]   # 512, 640, 768
KC = [f // P for f in FAN]                  # 4, 5, 6 contraction chunks
NBLK = SC // 512         # 4 token blocks for the hard phase

_cached = {}


@with_exitstack
def _emit(ctx: ExitStack, tc: tile.TileContext, io: dict):
    nc = tc.nc

    const = ctx.enter_context(tc.tile_pool(name="const", bufs=1))
    epool = ctx.enter_context(tc.tile_pool(name="epool", bufs=5))
    ohpool = ctx.enter_context(tc.tile_pool(name="ohpool", bufs=8))
    scr = ctx.enter_context(tc.tile_pool(name="scr", bufs=6))
    hout = ctx.enter_context(tc.tile_pool(name="hout", bufs=3))
    psum = ctx.enter_context(tc.tile_pool(name="psum", bufs=8, space="PSUM"))

    # ---------------- persistent SBUF tensors + input DMAs ----------------
    # Ordered by first consumption: hard phase (iotap/cb/idxb) first, then
    # main level 0 (xT/W0), then the rest.
    iotap = const.tile([P, 4], F32)
    nc.sync.dma_start(iotap[:], io["iotap"])
    cb = const.tile([P, L, 4, C], BF16)
    nc.sync.dma_start(cb[:], io["cb"])
    idxb = const.tile([P, L, NC_, SC], F16)
    for blk in range(NBLK):
        nc.sync.dma_start(idxb[:, 0, :, blk * 512:(blk + 1) * 512],
                          io["idxb"][:, 0, :, blk * 512:(blk + 1) * 512])
    for lvl in range(1, L):
        nc.sync.dma_start(idxb[:, lvl], io["idxb"][:, lvl])
    ones2 = const.tile([2, P], BF16)
    nc.sync.dma_start(ones2[:], io["ones2"])
    biasHL = const.tile([2, L, FOUT], BF16)
    nc.sync.dma_start(biasHL[:], io["biasHL"])
    cond = const.tile([P, 6, SC], BF16)
    nc.sync.dma_start(cond[:, 0:4, :], io["xT"])
    w0 = const.tile([P, KC[0], FOUT], BF16)
    nc.sync.dma_start(w0[:], io["W0T"])
    iotaf = const.tile([P, FOUT], F16)
    nc.sync.dma_start(iotaf[:], io["iotaf"])
    idxg = const.tile([P, L, NTILES * NC_], F32)
    nc.sync.dma_start(idxg[:], io["idxg"])
    w1 = const.tile([P, KC[1], FOUT], BF16)
    nc.sync.dma_start(w1[:], io["W1T"])
    w2 = const.tile([P, KC[2], FOUT], BF16)
    nc.sync.dma_start(w2[:], io["W2T"])
    ws = [w0, w1, w2]

    statZ = const.tile([P, NTILES, L * NC_], F32)
    statR2 = const.tile([P, NTILES, L * NC_], F32)
    statE = const.tile([P, NTILES, L * NC_], F32)

    repeat = int(os.environ.get("K_REPEAT", "1"))

    # Pre-touch const tiles so downstream instructions don't each carry a
    # DMA wait (walrus allows few sync-wait commands per instruction).
    touch = const.tile([2, 8], F32)
    nc.vector.tensor_copy(touch[:, 0:1], idxb[0:2, 0, 0, 0:1])
    nc.vector.tensor_copy(touch[:, 1:2], iotap[0:2, 0:1])
    nc.vector.tensor_copy(touch[:, 2:3], iotaf[0:2, 0:1])
    nc.vector.tensor_copy(touch[:, 3:4], idxg[0:2, 0, 0:1])

    # ---------------- hard phase: codeword lookups as matmuls -------------
    # psum accumulates the cumulative hard sums S1, S2, S3 per token block.
    # Level-outer order: the S1/S2 copy of one block overlaps the matmuls of
    # the other blocks, so the PE never waits on the copies.
    for _rep in range(repeat):
     phs = [psum.tile([P, 512], F32, tag="ps", name=f"ph{b}")
            for b in range(NBLK)]
     for lvl in range(L):
        for blk in range(NBLK):
            ph = phs[blk]
            for ch in range(4):
                for ncc in range(NC_):
                    oh = ohpool.tile([P, 512], BF16, tag="oh")
                    nc.vector.tensor_scalar(
                        out=oh[:],
                        in0=idxb[:, lvl, ncc, blk * 512:(blk + 1) * 512],
                        scalar1=iotap[:, ch:ch + 1], scalar2=None,
                        op0=OP.is_equal)
                    nc.tensor.matmul(
                        ph[32 * ncc:32 * ncc + 32, :],
                        cb[:, lvl, ch], oh[:],
                        start=(lvl == 0 and ch == 0),
                        stop=(lvl == L - 1 and ch == 3),
                        skip_group_check=True,
                        tile_position=(0, 32 * ncc))
            if lvl < L - 1:
                nc.scalar.copy(cond[:, 4 + lvl, blk * 512:(blk + 1) * 512],
                               ph[:])
            else:
                hs = hout.tile([P, 512], F32, tag="hs")
                nc.scalar.copy(hs[:], ph[:])
                nc.sync.dma_start(io["out_hard"][:, blk * 512:(blk + 1) * 512],
                                  hs[:])

     # ---------------- main: logits -> exp/Z -> R2 -> gather --------------
     for lvl in range(L):
        w = ws[lvl]
        for t in range(NTILES):
            Et = epool.tile([P, FOUT], F16, tag="E")
            for ncc in range(NC_):
                ps = psum.tile([P, 512], F32, tag="ps")
                for k in range(KC[lvl]):
                    nc.tensor.matmul(
                        ps[:], cond[:, k, t * P:(t + 1) * P],
                        w[:, k, ncc * 512:(ncc + 1) * 512],
                        start=(k == 0), stop=False)
                nc.tensor.matmul(
                    ps[:], ones2[:], biasHL[:, lvl, ncc * 512:(ncc + 1) * 512],
                    start=False, stop=True)
                sl = slice(ncc * 512, (ncc + 1) * 512)
                col = lvl * NC_ + ncc
                nc.scalar.activation(Et[:, sl], ps[:], AF.Exp,
                                     accum_out=statZ[:, t, col:col + 1])
                s2 = scr.tile([P, 512], F16, tag="s2")
                nc.vector.scalar_tensor_tensor(
                    out=s2[:], in0=iotaf[:, sl],
                    scalar=idxg[:, lvl, t * NC_ + ncc:t * NC_ + ncc + 1],
                    in1=Et[:, sl],
                    op0=OP.is_equal, op1=OP.mult,
                    accum_out=statE[:, t, col:col + 1])
                s1 = scr.tile([P, 512], F16, tag="s1")
                nc.vector.scalar_tensor_tensor(
                    out=s1[:], in0=Et[:, sl], scalar=1.0, in1=ps[:],
                    op0=OP.mult, op1=OP.mult,
                    accum_out=statR2[:, t, col:col + 1])

    # ---------------- finalize: lp / entropy (4 batches of 4 tiles) -------
    lnZ = const.tile([P, NTILES, L * NC_], F32)
    lnE = const.tile([P, NTILES, L * NC_], F32)
    rZ = const.tile([P, NTILES, L * NC_], F32)
    pe = const.tile([P, NTILES, L * NC_], F32)
    entc = const.tile([P, NTILES, L * NC_], F32)
    lpc = const.tile([P, NTILES, L * NC_], F32)
    lp = const.tile([P, NTILES], F32)
    ent = const.tile([P, NTILES], F32)
    for b0 in range(0, NTILES, 4):
        ts_ = slice(b0, b0 + 4)
        nc.scalar.activation(lnZ[:, ts_], statZ[:, ts_], AF.Ln)
        nc.scalar.activation(lnE[:, ts_], statE[:, ts_], AF.Ln)
        nc.vector.reciprocal(rZ[:, ts_], statZ[:, ts_])
        nc.vector.tensor_tensor(out=pe[:, ts_], in0=statR2[:, ts_],
                                in1=rZ[:, ts_], op=OP.mult)
        nc.vector.tensor_tensor(out=entc[:, ts_], in0=lnZ[:, ts_],
                                in1=pe[:, ts_], op=OP.subtract)
        nc.vector.tensor_tensor(out=lpc[:, ts_], in0=lnE[:, ts_],
                                in1=lnZ[:, ts_], op=OP.subtract)
        nc.vector.reduce_sum(lp[:, ts_], lpc[:, ts_],
                             axis=mybir.AxisListType.X)
        nc.vector.reduce_sum(ent[:, ts_], entc[:, ts_],
                             axis=mybir.AxisListType.X)
    nc.sync.dma_start(io["out_lp"], lp[:])
    nc.sync.dma_start(io["out_ent"], ent[:])


def _build():
    if "nc" in _cached:
        return _cached["nc"]
    nc = bacc.Bacc("TRN2", debug=False, num_devices=NCORES)
    io = {}
    io["xT"] = nc.dram_tensor("xT", [P, 4, SC], BF16, kind="ExternalInput").ap()
    for l in range(L):
        io[f"W{l}T"] = nc.dram_tensor(
            f"W{l}T", [P, KC[l], FOUT], BF16, kind="ExternalInput").ap()
    io["cb"] = nc.dram_tensor("cb", [P, L, 4, C], BF16,
                              kind="ExternalInput").ap()
    io["iotap"] = nc.dram_tensor("iotap", [P, 4], F32,
                                 kind="ExternalInput").ap()
    io["idxb"] = nc.dram_tensor("idxb", [P, L, NC_, SC], F16,
                                kind="ExternalInput").ap()
    io["iotaf"] = nc.dram_tensor("iotaf", [P, FOUT], F16,
                                 kind="ExternalInput").ap()
    io["idxg"] = nc.dram_tensor("idxg", [P, L, NTILES * NC_], F32,
                                kind="ExternalInput").ap()
    io["biasHL"] = nc.dram_tensor("biasHL", [2, L, FOUT], BF16,
                                  kind="ExternalInput").ap()
    io["ones2"] = nc.dram_tensor("ones2", [2, P], BF16,
                                 kind="ExternalInput").ap()
    io["out_hard"] = nc.dram_tensor("out_hard", [P, SC], F32,
                                    kind="ExternalOutput").ap()
    io["out_lp"] = nc.dram_tensor("out_lp", [P, NTILES], F32,
                                  kind="ExternalOutput").ap()
    io["out_ent"] = nc.dram_tensor("out_ent", [P, NTILES], F32,
                                   kind="ExternalOutput").ap()
    with tile.TileContext(nc) as tc:
        _emit(tc, io)
    nc.compile()
    _cached["nc"] = nc
    return nc


def _prep_inputs(x, comms, codebook, W0, b0, W1, b1, W2, b2):
    bf = ml_dtypes.bfloat16
    x = np.ascontiguousarray(np.asarray(x, np.float32).reshape(S, H))
    comms = np.ascontiguousarray(np.asarray(comms).reshape(S, NC_, L))
    codebook = np.asarray(codebook, np.float32)
    Ws = [np.asarray(W0, np.float32), np.asarray(W1, np.float32),
          np.asarray(W2, np.float32).copy()]
    bs = np.stack([np.asarray(b, np.float32) for b in (b0, b1, b2)])

    # cumulative-hard transform: cond chunks are [x, S1, S2] with
    # S1 = h0, S2 = h0 + h1, so W2's h0-block absorbs -h1-block.
    Ws[2][:, H:H + 128] -= Ws[2][:, H + 128:H + 256]

    shared = {}
    for l in range(L):
        wt = Ws[l].T.astype(bf)                      # [fan, 2048]
        shared[f"W{l}T"] = np.ascontiguousarray(
            wt.reshape(KC[l], P, FOUT).transpose(1, 0, 2))
    b_hi = bs.astype(bf)
    b_lo = (bs - b_hi.astype(np.float32)).astype(bf)
    shared["biasHL"] = np.ascontiguousarray(np.stack([b_hi, b_lo]))
    shared["ones2"] = np.ones((2, P), bf)
    cb_hi = codebook.astype(bf)
    cbd = np.zeros((P, L, 4, C), bf)
    for l in range(L):
        for ch in range(4):
            cbd[:, l, ch] = cb_hi[l, ch * P:(ch + 1) * P]
    shared["cb"] = cbd
    shared["iotap"] = (np.arange(P)[:, None]
                       + P * np.arange(4)[None, :]).astype(np.float32)
    shared["iotaf"] = np.ascontiguousarray(np.broadcast_to(
        np.arange(FOUT, dtype=np.float16), (P, FOUT)))

    in_maps = []
    for c in range(NCORES):
        tok = slice(c * SC, (c + 1) * SC)
        xc = x[tok]                                   # [2048, 512]
        xT = np.ascontiguousarray(
            xc.T.astype(bf).reshape(4, P, SC).transpose(1, 0, 2))
        cm = comms[tok]                               # [2048, NC, L]
        idxb = np.ascontiguousarray(np.broadcast_to(
            cm.transpose(2, 1, 0).astype(np.float16)[None],
            (P, L, NC_, SC)))
        # idxg[p, l, t*NC+nc] = nc*512 + comms[t*128+p, nc, l]
        cmt = cm.reshape(NTILES, P, NC_, L)
        idxg = (cmt.transpose(1, 3, 0, 2).astype(np.float32)
                + (np.arange(NC_) * V).astype(np.float32)[None, None, None, :])
        idxg = np.ascontiguousarray(idxg.reshape(P, L, NTILES * NC_))
        m = dict(shared)
        m["xT"] = xT
        m["idxb"] = idxb
        m["idxg"] = idxg.astype(np.float32)
        in_maps.append(m)
    return in_maps


def _run(in_maps, **kwargs):
    nc = _build()
    return bass_utils.run_bass_kernel_spmd(
        nc, in_maps, core_ids=list(range(NCORES)), **kwargs)


def _lo_correction(comms, codebook):
    # fp32 - bf16(codebook): the part the on-device bf16 matmul cannot see.
    cb = np.asarray(codebook, np.float32)
    lo = cb - cb.astype(ml_dtypes.bfloat16).astype(np.float32)  # [L, V, C]
    cm = np.asarray(comms).reshape(S, NC_, L)
    out = np.zeros((S, NC_, C), np.float32)
    for l in range(L):
        out += lo[l][cm[:, :, l]]
    return out.reshape(S, NC_ * C)


def _postprocess(results, locorr=None):
    comm = np.concatenate(
        [np.asarray(r["out_hard"], np.float32).T for r in results], axis=0)
    if locorr is not None:
        comm = comm + locorr
    lp = np.concatenate(
        [np.asarray(r["out_lp"], np.float32).T.reshape(-1) for r in results])
    ent = np.concatenate(
        [np.asarray(r["out_ent"], np.float32).T.reshape(-1) for r in results])
    return (comm, lp.reshape(B, T, N), ent.reshape(B, T, N))


def kernel(x, comms, codebook, W0, b0, W1, b1, W2, b2):
    in_maps = _prep_inputs(x, comms, codebook, W0, b0, W1, b1, W2, b2)
    locorr = _lo_correction(comms, codebook)
    res = _run(in_maps)
    return _postprocess(res.results, locorr)


# revision 19
# speedup vs baseline: 1.0064x; 1.0064x over previous
"""Trainium2 Bass kernel for nn_AimComms (vq_codebook).

Data-parallel over 8 NeuronCores: tokens (B*T*N = 16384) sharded 2048/core.

Per-core layout (S1): tokens on partitions for the softmax pipeline,
features on partitions for the cond/weight matmuls.

Key tricks:
- All matmuls bf16 (fp32 PE matmul is 4x slower); bias added exactly via a
  K=2 matmul against [b_hi; b_lo] (bf16 hi/lo split), codebook likewise.
- Codeword lookup = one-hot (device-built via DVE is_equal on fp16
  replicated indices) x codebook matmul, col-packed M=32 (tile_position).
- Cumulative-hard: PSUM accumulates h0, h0+h1, h0+h1+h2 across levels in one
  accumulation group (W2 is host-transformed so cond uses cumulative sums);
  the final PSUM state is comm_output directly.
- Softmax Z for free via ACT Exp accum_out; entropy dot-product sum(E*s) and
  the log-prob gather (sum(E * (iota==idx))) via fused scalar_tensor_tensor
  with accum_out on DVE.
- log p = s_idx - logZ computed as ln(E_idx) - ln(Z).
"""

import os
from contextlib import ExitStack

import numpy as np
import ml_dtypes

import concourse.bass as bass
import concourse.tile as tile
import concourse.mybir as mybir
from concourse import bacc, bass_utils
from concourse._compat import with_exitstack

BF16 = mybir.dt.bfloat16
F16 = mybir.dt.float16
F32 = mybir.dt.float32
AF = mybir.ActivationFunctionType
OP = mybir.AluOpType

P = 128
NCORES = 8
B, T, N = 32, 64, 8
NC_, L, C, V, H = 4, 3, 32, 512, 512
S = B * T * N            # 16384 tokens total
SC = S // NCORES         # 2048 tokens per core
NTILES = SC // P         # 16 token tiles per core
FOUT = NC_ * V           # 2048 logits per level
FAN = [H + l * NC_ * C for l in range(L)]   # 512, 640, 768
KC = [f // P for f in FAN]                  # 4, 5, 6 contraction chunks
NBLK = SC // 512         # 4 token blocks for the hard phase

_cached = {}


@with_exitstack
def _emit(ctx: ExitStack, tc: tile.TileContext, io: dict):
    nc = tc.nc

    const = ctx.enter_context(tc.tile_pool(name="const", bufs=1))
    epool = ctx.enter_context(tc.tile_pool(name="epool", bufs=5))
    ohpool = ctx.enter_context(tc.tile_pool(name="ohpool", bufs=8))
    scr = ctx.enter_context(tc.tile_pool(name="scr", bufs=6))
    hout = ctx.enter_context(tc.tile_pool(name="hout", bufs=3))
    psum = ctx.enter_context(tc.tile_pool(name="psum", bufs=8, space="PSUM"))

    # ---------------- persistent SBUF tensors + input DMAs ----------------
    # Ordered by first consumption: hard phase (iotap/cb/idxb) first, then
    # main level 0 (xT/W0), then the rest.
    cb = const.tile([P, L, 4, C], BF16)
    nc.sync.dma_start(cb[:], io["cb"])
    iotap = const.tile([P, 4], F32)
    nc.sync.dma_start(iotap[:], io["iotap"])
    idxb = const.tile([P, L, NC_, SC], F16)
    for blk in range(NBLK):
        nc.sync.dma_start(idxb[:, 0, :, blk * 512:(blk + 1) * 512],
                          io["idxb"][:, 0, :, blk * 512:(blk + 1) * 512])
    for lvl in range(1, L):
        nc.sync.dma_start(idxb[:, lvl], io["idxb"][:, lvl])
    ones2 = const.tile([2, P], BF16)
    nc.sync.dma_start(ones2[:], io["ones2"])
    biasHL = const.tile([2, L, FOUT], BF16)
    nc.sync.dma_start(biasHL[:], io["biasHL"])
    cond = const.tile([P, 6, SC], BF16)
    nc.sync.dma_start(cond[:, 0:4, :], io["xT"])
    w0 = const.tile([P, KC[0], FOUT], BF16)
    nc.sync.dma_start(w0[:], io["W0T"])
    iotaf = const.tile([P, FOUT], F16)
    nc.sync.dma_start(iotaf[:], io["iotaf"])
    idxg = const.tile([P, L, NTILES * NC_], F32)
    nc.sync.dma_start(idxg[:], io["idxg"])
    w1 = const.tile([P, KC[1], FOUT], BF16)
    nc.sync.dma_start(w1[:], io["W1T"])
    w2 = const.tile([P, KC[2], FOUT], BF16)
    nc.sync.dma_start(w2[:], io["W2T"])
    ws = [w0, w1, w2]

    statZ = const.tile([P, NTILES, L * NC_], F32)
    statR2 = const.tile([P, NTILES, L * NC_], F32)
    statE = const.tile([P, NTILES, L * NC_], F32)

    repeat = int(os.environ.get("K_REPEAT", "1"))

    # Pre-touch const tiles so downstream instructions don't each carry a
    # DMA wait (walrus allows few sync-wait commands per instruction).
    touch = const.tile([2, 8], F32)
    nc.vector.tensor_copy(touch[:, 0:1], idxb[0:2, 0, 0, 0:1])
    nc.vector.tensor_copy(touch[:, 1:2], iotap[0:2, 0:1])

    # ---------------- hard phase: codeword lookups as matmuls -------------
    # psum accumulates the cumulative hard sums S1, S2, S3 per token block.
    # Level-outer order: the S1/S2 copy of one block overlaps the matmuls of
    # the other blocks, so the PE never waits on the copies.
    for _rep in range(repeat):
     phs = [psum.tile([P, 512], F32, tag="ps", name=f"ph{b}")
            for b in range(NBLK)]
     for lvl in range(L):
        for blk in range(NBLK):
            ph = phs[blk]
            for ch in range(4):
                for ncc in range(NC_):
                    oh = ohpool.tile([P, 512], BF16, tag="oh")
                    nc.vector.tensor_scalar(
                        out=oh[:],
                        in0=idxb[:, lvl, ncc, blk * 512:(blk + 1) * 512],
                        scalar1=iotap[:, ch:ch + 1], scalar2=None,
                        op0=OP.is_equal)
                    nc.tensor.matmul(
                        ph[32 * ncc:32 * ncc + 32, :],
                        cb[:, lvl, ch], oh[:],
                        start=(lvl == 0 and ch == 0),
                        stop=(lvl == L - 1 and ch == 3),
                        skip_group_check=True,
                        tile_position=(0, 32 * ncc))
            if lvl < L - 1:
                nc.scalar.copy(cond[:, 4 + lvl, blk * 512:(blk + 1) * 512],
                               ph[:])
            else:
                hs = hout.tile([P, 512], F32, tag="hs")
                nc.scalar.copy(hs[:], ph[:])
                nc.sync.dma_start(io["out_hard"][:, blk * 512:(blk + 1) * 512],
                                  hs[:])

     # consolidate the DMA waits for the main-loop DVE constants here, so
     # the hard-phase one-hots above never queue behind them
     nc.vector.tensor_copy(touch[:, 2:3], iotaf[0:2, 0:1])
     nc.vector.tensor_copy(touch[:, 3:4], idxg[0:2, 0, 0:1])

     # ---------------- main: logits -> exp/Z -> R2 -> gather --------------
     for lvl in range(L):
        w = ws[lvl]
        for t in range(NTILES):
            Et = epool.tile([P, FOUT], F16, tag="E")
            for ncc in range(NC_):
                ps = psum.tile([P, 512], F32, tag="ps")
                for k in range(KC[lvl]):
                    nc.tensor.matmul(
                        ps[:], cond[:, k, t * P:(t + 1) * P],
                        w[:, k, ncc * 512:(ncc + 1) * 512],
                        start=(k == 0), stop=False)
                nc.tensor.matmul(
                    ps[:], ones2[:], biasHL[:, lvl, ncc * 512:(ncc + 1) * 512],
                    start=False, stop=True)
                sl = slice(ncc * 512, (ncc + 1) * 512)
                col = lvl * NC_ + ncc
                nc.scalar.activation(Et[:, sl], ps[:], AF.Exp,
                                     accum_out=statZ[:, t, col:col + 1])
                s2 = scr.tile([P, 512], F16, tag="s2")
                nc.vector.scalar_tensor_tensor(
                    out=s2[:], in0=iotaf[:, sl],
                    scalar=idxg[:, lvl, t * NC_ + ncc:t * NC_ + ncc + 1],
                    in1=Et[:, sl],
                    op0=OP.is_equal, op1=OP.mult,
                    accum_out=statE[:, t, col:col + 1])
                s1 = scr.tile([P, 512], F16, tag="s1")
                nc.vector.scalar_tensor_tensor(
                    out=s1[:], in0=Et[:, sl], scalar=1.0, in1=ps[:],
                    op0=OP.mult, op1=OP.mult,
                    accum_out=statR2[:, t, col:col + 1])

    # ---------------- finalize: lp / entropy (4 batches of 4 tiles) -------
    lnZ = const.tile([P, NTILES, L * NC_], F32)
    lnE = const.tile([P, NTILES, L * NC_], F32)
    rZ = const.tile([P, NTILES, L * NC_], F32)
    pe = const.tile([P, NTILES, L * NC_], F32)
    entc = const.tile([P, NTILES, L * NC_], F32)
    lpc = const.tile([P, NTILES, L * NC_], F32)
    lpent = const.tile([P, 2, NTILES], F32)
    lp = lpent[:, 0]
    ent = lpent[:, 1]
    for b0 in range(0, NTILES, 4):
        ts_ = slice(b0, b0 + 4)
        nc.scalar.activation(lnZ[:, ts_], statZ[:, ts_], AF.Ln)
        nc.scalar.activation(lnE[:, ts_], statE[:, ts_], AF.Ln)
        nc.vector.reciprocal(rZ[:, ts_], statZ[:, ts_])
        nc.vector.tensor_tensor(out=pe[:, ts_], in0=statR2[:, ts_],
                                in1=rZ[:, ts_], op=OP.mult)
        nc.vector.tensor_tensor(out=entc[:, ts_], in0=lnZ[:, ts_],
                                in1=pe[:, ts_], op=OP.subtract)
        nc.vector.tensor_tensor(out=lpc[:, ts_], in0=lnE[:, ts_],
                                in1=lnZ[:, ts_], op=OP.subtract)
        nc.vector.reduce_sum(lp[:, ts_], lpc[:, ts_],
                             axis=mybir.AxisListType.X)
        nc.vector.reduce_sum(ent[:, ts_], entc[:, ts_],
                             axis=mybir.AxisListType.X)
    nc.sync.dma_start(io["out_lpent"], lpent[:])


def _build():
    if "nc" in _cached:
        return _cached["nc"]
    nc = bacc.Bacc("TRN2", debug=False, num_devices=NCORES)
    io = {}
    io["xT"] = nc.dram_tensor("xT", [P, 4, SC], BF16, kind="ExternalInput").ap()
    for l in range(L):
        io[f"W{l}T"] = nc.dram_tensor(
            f"W{l}T", [P, KC[l], FOUT], BF16, kind="ExternalInput").ap()
    io["cb"] = nc.dram_tensor("cb", [P, L, 4, C], BF16,
                              kind="ExternalInput").ap()
    io["iotap"] = nc.dram_tensor("iotap", [P, 4], F32,
                                 kind="ExternalInput").ap()
    io["idxb"] = nc.dram_tensor("idxb", [P, L, NC_, SC], F16,
                                kind="ExternalInput").ap()
    io["iotaf"] = nc.dram_tensor("iotaf", [P, FOUT], F16,
                                 kind="ExternalInput").ap()
    io["idxg"] = nc.dram_tensor("idxg", [P, L, NTILES * NC_], F32,
                                kind="ExternalInput").ap()
    io["biasHL"] = nc.dram_tensor("biasHL", [2, L, FOUT], BF16,
                                  kind="ExternalInput").ap()
    io["ones2"] = nc.dram_tensor("ones2", [2, P], BF16,
                                 kind="ExternalInput").ap()
    io["out_hard"] = nc.dram_tensor("out_hard", [P, SC], F32,
                                    kind="ExternalOutput").ap()
    io["out_lpent"] = nc.dram_tensor("out_lpent", [P, 2, NTILES], F32,
                                     kind="ExternalOutput").ap()
    with tile.TileContext(nc) as tc:
        _emit(tc, io)
    nc.compile()
    _cached["nc"] = nc
    return nc


def _prep_inputs(x, comms, codebook, W0, b0, W1, b1, W2, b2):
    bf = ml_dtypes.bfloat16
    x = np.ascontiguousarray(np.asarray(x, np.float32).reshape(S, H))
    comms = np.ascontiguousarray(np.asarray(comms).reshape(S, NC_, L))
    codebook = np.asarray(codebook, np.float32)
    Ws = [np.asarray(W0, np.float32), np.asarray(W1, np.float32),
          np.asarray(W2, np.float32).copy()]
    bs = np.stack([np.asarray(b, np.float32) for b in (b0, b1, b2)])

    # cumulative-hard transform: cond chunks are [x, S1, S2] with
    # S1 = h0, S2 = h0 + h1, so W2's h0-block absorbs -h1-block.
    Ws[2][:, H:H + 128] -= Ws[2][:, H + 128:H + 256]

    shared = {}
    for l in range(L):
        wt = Ws[l].T.astype(bf)                      # [fan, 2048]
        shared[f"W{l}T"] = np.ascontiguousarray(
            wt.reshape(KC[l], P, FOUT).transpose(1, 0, 2))
    b_hi = bs.astype(bf)
    b_lo = (bs - b_hi.astype(np.float32)).astype(bf)
    shared["biasHL"] = np.ascontiguousarray(np.stack([b_hi, b_lo]))
    shared["ones2"] = np.ones((2, P), bf)
    cb_hi = codebook.astype(bf)
    cbd = np.zeros((P, L, 4, C), bf)
    for l in range(L):
        for ch in range(4):
            cbd[:, l, ch] = cb_hi[l, ch * P:(ch + 1) * P]
    shared["cb"] = cbd
    shared["iotap"] = (np.arange(P)[:, None]
                       + P * np.arange(4)[None, :]).astype(np.float32)
    shared["iotaf"] = np.ascontiguousarray(np.broadcast_to(
        np.arange(FOUT, dtype=np.float16), (P, FOUT)))

    in_maps = []
    for c in range(NCORES):
        tok = slice(c * SC, (c + 1) * SC)
        xc = x[tok]                                   # [2048, 512]
        xT = np.ascontiguousarray(
            xc.T.astype(bf).reshape(4, P, SC).transpose(1, 0, 2))
        cm = comms[tok]                               # [2048, NC, L]
        idxb = np.ascontiguousarray(np.broadcast_to(
            cm.transpose(2, 1, 0).astype(np.float16)[None],
            (P, L, NC_, SC)))
        # idxg[p, l, t*NC+nc] = nc*512 + comms[t*128+p, nc, l]
        cmt = cm.reshape(NTILES, P, NC_, L)
        idxg = (cmt.transpose(1, 3, 0, 2).astype(np.float32)
                + (np.arange(NC_) * V).astype(np.float32)[None, None, None, :])
        idxg = np.ascontiguousarray(idxg.reshape(P, L, NTILES * NC_))
        m = dict(shared)
        m["xT"] = xT
        m["idxb"] = idxb
        m["idxg"] = idxg.astype(np.float32)
        in_maps.append(m)
    return in_maps


def _run(in_maps, **kwargs):
    nc = _build()
    return bass_utils.run_bass_kernel_spmd(
        nc, in_maps, core_ids=list(range(NCORES)), **kwargs)


def _lo_correction(comms, codebook):
    # fp32 - bf16(codebook): the part the on-device bf16 matmul cannot see.
    cb = np.asarray(codebook, np.float32)
    lo = cb - cb.astype(ml_dtypes.bfloat16).astype(np.float32)  # [L, V, C]
    cm = np.asarray(comms).reshape(S, NC_, L)
    out = np.zeros((S, NC_, C), np.float32)
    for l in range(L):
        out += lo[l][cm[:, :, l]]
    return out.reshape(S, NC_ * C)


def _postprocess(results, locorr=None):
    comm = np.concatenate(
        [np.asarray(r["out_hard"], np.float32).T for r in results], axis=0)
    if locorr is not None:
        comm = comm + locorr
    lp = np.concatenate(
        [np.asarray(r["out_lpent"], np.float32)[:, 0].T.reshape(-1)
         for r in results])
    ent = np.concatenate(
        [np.asarray(r["out_lpent"], np.float32)[:, 1].T.reshape(-1)
         for r in results])
    return (comm, lp.reshape(B, T, N), ent.reshape(B, T, N))


def kernel(x, comms, codebook, W0, b0, W1, b1, W2, b2):
    in_maps = _prep_inputs(x, comms, codebook, W0, b0, W1, b1, W2, b2)
    locorr = _lo_correction(comms, codebook)
    res = _run(in_maps)
    return _postprocess(res.results, locorr)


# revision 21
# speedup vs baseline: 1.0068x; 1.0004x over previous
"""Trainium2 Bass kernel for nn_AimComms (vq_codebook).

Data-parallel over 8 NeuronCores: tokens (B*T*N = 16384) sharded 2048/core.

Per-core layout (S1): tokens on partitions for the softmax pipeline,
features on partitions for the cond/weight matmuls.

Key tricks:
- All matmuls bf16 (fp32 PE matmul is 4x slower); bias added exactly via a
  K=2 matmul against [b_hi; b_lo] (bf16 hi/lo split), codebook likewise.
- Codeword lookup = one-hot (device-built via DVE is_equal on fp16
  replicated indices) x codebook matmul, col-packed M=32 (tile_position).
- Cumulative-hard: PSUM accumulates h0, h0+h1, h0+h1+h2 across levels in one
  accumulation group (W2 is host-transformed so cond uses cumulative sums);
  the final PSUM state is comm_output directly.
- Softmax Z for free via ACT Exp accum_out; entropy dot-product sum(E*s) and
  the log-prob gather (sum(E * (iota==idx))) via fused scalar_tensor_tensor
  with accum_out on DVE.
- log p = s_idx - logZ computed as ln(E_idx) - ln(Z).
"""

import os
from contextlib import ExitStack

import numpy as np
import ml_dtypes

import concourse.bass as bass
import concourse.tile as tile
import concourse.mybir as mybir
from concourse import bacc, bass_utils
from concourse._compat import with_exitstack

BF16 = mybir.dt.bfloat16
F16 = mybir.dt.float16
F32 = mybir.dt.float32
AF = mybir.ActivationFunctionType
OP = mybir.AluOpType

P = 128
NCORES = 8
B, T, N = 32, 64, 8
NC_, L, C, V, H = 4, 3, 32, 512, 512
S = B * T * N            # 16384 tokens total
SC = S // NCORES         # 2048 tokens per core
NTILES = SC // P         # 16 token tiles per core
FOUT = NC_ * V           # 2048 logits per level
FAN = [H + l * NC_ * C for l in range(L)]   # 512, 640, 768
KC = [f // P for f in FAN]                  # 4, 5, 6 contraction chunks
NBLK = SC // 512         # 4 token blocks for the hard phase

_cached = {}


@with_exitstack
def _emit(ctx: ExitStack, tc: tile.TileContext, io: dict):
    nc = tc.nc

    const = ctx.enter_context(tc.tile_pool(name="const", bufs=1))
    epool = ctx.enter_context(tc.tile_pool(name="epool", bufs=5))
    ohpool = ctx.enter_context(tc.tile_pool(name="ohpool", bufs=8))
    scr = ctx.enter_context(tc.tile_pool(name="scr", bufs=6))
    hout = ctx.enter_context(tc.tile_pool(name="hout", bufs=3))
    psum = ctx.enter_context(tc.tile_pool(name="psum", bufs=8, space="PSUM"))

    # ---------------- persistent SBUF tensors + input DMAs ----------------
    # Ordered by first consumption: hard phase (iotap/cb/idxb) first, then
    # main level 0 (xT/W0), then the rest.
    cb = const.tile([P, L, 4, C], BF16)
    nc.sync.dma_start(cb[:], io["cb"])
    iotap = const.tile([P, 4], F32)
    nc.sync.dma_start(iotap[:], io["iotap"])
    idxb = const.tile([P, L, NC_, SC], F16)
    for blk in range(NBLK):
        nc.sync.dma_start(idxb[:, 0, :, blk * 512:(blk + 1) * 512],
                          io["idxb"][:, 0, :, blk * 512:(blk + 1) * 512])
    for lvl in range(1, L):
        nc.sync.dma_start(idxb[:, lvl], io["idxb"][:, lvl])
    ones2 = const.tile([2, P], BF16)
    nc.sync.dma_start(ones2[:], io["ones2"])
    biasHL = const.tile([2, L, FOUT], BF16)
    nc.sync.dma_start(biasHL[:], io["biasHL"])
    cond = const.tile([P, 6, SC], BF16)
    nc.sync.dma_start(cond[:, 0:4, :], io["xT"])
    w0 = const.tile([P, KC[0], FOUT], BF16)
    nc.sync.dma_start(w0[:], io["W0T"])
    iotaf = const.tile([P, FOUT], F16)
    nc.sync.dma_start(iotaf[:], io["iotaf"])
    idxg = const.tile([P, L, NTILES * NC_], F32)
    nc.sync.dma_start(idxg[:], io["idxg"])
    w1 = const.tile([P, KC[1], FOUT], BF16)
    nc.sync.dma_start(w1[:], io["W1T"])
    w2 = const.tile([P, KC[2], FOUT], BF16)
    nc.sync.dma_start(w2[:], io["W2T"])
    ws = [w0, w1, w2]

    statZ = const.tile([P, NTILES, L * NC_], F32)
    statR2 = const.tile([P, NTILES, L * NC_], F32)
    statE = const.tile([P, NTILES, L * NC_], F32)

    repeat = int(os.environ.get("K_REPEAT", "1"))

    # Pre-touch const tiles so downstream instructions don't each carry a
    # DMA wait (walrus allows few sync-wait commands per instruction).
    touch = const.tile([2, 8], F32)
    nc.vector.tensor_copy(touch[:, 0:1], idxb[0:2, 0, 0, 0:1])
    nc.vector.tensor_copy(touch[:, 1:2], iotap[0:2, 0:1])

    # ---------------- hard phase: codeword lookups as matmuls -------------
    # psum accumulates the cumulative hard sums S1, S2, S3 per token block.
    # Level-outer order: the S1/S2 copy of one block overlaps the matmuls of
    # the other blocks, so the PE never waits on the copies.
    for _rep in range(repeat):
     phs = [psum.tile([P, 512], F32, tag="ps", name=f"ph{b}")
            for b in range(NBLK)]
     for lvl in range(L):
        for blk in range(NBLK):
            ph = phs[blk]
            for ch in range(4):
                for ncc in range(NC_):
                    oh = ohpool.tile([P, 512], BF16, tag="oh")
                    nc.vector.tensor_scalar(
                        out=oh[:],
                        in0=idxb[:, lvl, ncc, blk * 512:(blk + 1) * 512],
                        scalar1=iotap[:, ch:ch + 1], scalar2=None,
                        op0=OP.is_equal)
                    nc.tensor.matmul(
                        ph[32 * ncc:32 * ncc + 32, :],
                        cb[:, lvl, ch], oh[:],
                        start=(lvl == 0 and ch == 0),
                        stop=(lvl == L - 1 and ch == 3),
                        skip_group_check=True,
                        tile_position=(0, 32 * ncc))
            if lvl < L - 1:
                nc.scalar.copy(cond[:, 4 + lvl, blk * 512:(blk + 1) * 512],
                               ph[:])
            else:
                hs = hout.tile([P, 512], F32, tag="hs")
                nc.scalar.copy(hs[:], ph[:])
                nc.sync.dma_start(io["out_hard"][:, blk * 512:(blk + 1) * 512],
                                  hs[:])

     # consolidate the DMA waits for the main-loop DVE constants here, so
     # the hard-phase one-hots above never queue behind them
     nc.vector.tensor_copy(touch[:, 2:3], iotaf[0:2, 0:1])
     nc.vector.tensor_copy(touch[:, 3:4], idxg[0:2, 0, 0:1])

     # ---------------- main: logits -> exp/Z -> R2 -> gather --------------
     for lvl in range(L):
        w = ws[lvl]
        for t in range(NTILES):
            Et = epool.tile([P, FOUT], F16, tag="E")
            for ncc in range(NC_):
                ps = psum.tile([P, 512], F32, tag="ps")
                # bias first: operands always resident, so its dispatch (and
                # the bank-WAR event-semaphore) hoists under the previous
                # group's streaming instead of stalling the PE.
                nc.tensor.matmul(
                    ps[:], ones2[:], biasHL[:, lvl, ncc * 512:(ncc + 1) * 512],
                    start=True, stop=False)
                for k in range(KC[lvl]):
                    nc.tensor.matmul(
                        ps[:], cond[:, k, t * P:(t + 1) * P],
                        w[:, k, ncc * 512:(ncc + 1) * 512],
                        start=False, stop=(k == KC[lvl] - 1))
                sl = slice(ncc * 512, (ncc + 1) * 512)
                col = lvl * NC_ + ncc
                nc.scalar.activation(Et[:, sl], ps[:], AF.Exp,
                                     accum_out=statZ[:, t, col:col + 1])
                s2 = scr.tile([P, 512], F16, tag="s2")
                nc.vector.scalar_tensor_tensor(
                    out=s2[:], in0=iotaf[:, sl],
                    scalar=idxg[:, lvl, t * NC_ + ncc:t * NC_ + ncc + 1],
                    in1=Et[:, sl],
                    op0=OP.is_equal, op1=OP.mult,
                    accum_out=statE[:, t, col:col + 1])
                s1 = scr.tile([P, 512], F16, tag="s1")
                nc.vector.scalar_tensor_tensor(
                    out=s1[:], in0=Et[:, sl], scalar=1.0, in1=ps[:],
                    op0=OP.mult, op1=OP.mult,
                    accum_out=statR2[:, t, col:col + 1])

    # ---------------- finalize: lp / entropy (4 batches of 4 tiles) -------
    lnZ = const.tile([P, NTILES, L * NC_], F32)
    lnE = const.tile([P, NTILES, L * NC_], F32)
    rZ = const.tile([P, NTILES, L * NC_], F32)
    pe = const.tile([P, NTILES, L * NC_], F32)
    entc = const.tile([P, NTILES, L * NC_], F32)
    lpc = const.tile([P, NTILES, L * NC_], F32)
    lpent = const.tile([P, 2, NTILES], F32)
    lp = lpent[:, 0]
    ent = lpent[:, 1]
    for b0 in range(0, NTILES, 4):
        ts_ = slice(b0, b0 + 4)
        nc.scalar.activation(lnZ[:, ts_], statZ[:, ts_], AF.Ln)
        nc.scalar.activation(lnE[:, ts_], statE[:, ts_], AF.Ln)
        nc.vector.reciprocal(rZ[:, ts_], statZ[:, ts_])
        nc.vector.tensor_tensor(out=pe[:, ts_], in0=statR2[:, ts_],
                                in1=rZ[:, ts_], op=OP.mult)
        nc.vector.tensor_tensor(out=entc[:, ts_], in0=lnZ[:, ts_],
                                in1=pe[:, ts_], op=OP.subtract)
        nc.vector.tensor_tensor(out=lpc[:, ts_], in0=lnE[:, ts_],
                                in1=lnZ[:, ts_], op=OP.subtract)
        nc.vector.reduce_sum(lp[:, ts_], lpc[:, ts_],
                             axis=mybir.AxisListType.X)
        nc.vector.reduce_sum(ent[:, ts_], entc[:, ts_],
                             axis=mybir.AxisListType.X)
        nc.sync.dma_start(io["out_lpent"][:, :, ts_], lpent[:, :, ts_])


def _build():
    if "nc" in _cached:
        return _cached["nc"]
    nc = bacc.Bacc("TRN2", debug=False, num_devices=NCORES)
    io = {}
    io["xT"] = nc.dram_tensor("xT", [P, 4, SC], BF16, kind="ExternalInput").ap()
    for l in range(L):
        io[f"W{l}T"] = nc.dram_tensor(
            f"W{l}T", [P, KC[l], FOUT], BF16, kind="ExternalInput").ap()
    io["cb"] = nc.dram_tensor("cb", [P, L, 4, C], BF16,
                              kind="ExternalInput").ap()
    io["iotap"] = nc.dram_tensor("iotap", [P, 4], F32,
                                 kind="ExternalInput").ap()
    io["idxb"] = nc.dram_tensor("idxb", [P, L, NC_, SC], F16,
                                kind="ExternalInput").ap()
    io["iotaf"] = nc.dram_tensor("iotaf", [P, FOUT], F16,
                                 kind="ExternalInput").ap()
    io["idxg"] = nc.dram_tensor("idxg", [P, L, NTILES * NC_], F32,
                                kind="ExternalInput").ap()
    io["biasHL"] = nc.dram_tensor("biasHL", [2, L, FOUT], BF16,
                                  kind="ExternalInput").ap()
    io["ones2"] = nc.dram_tensor("ones2", [2, P], BF16,
                                 kind="ExternalInput").ap()
    io["out_hard"] = nc.dram_tensor("out_hard", [P, SC], F32,
                                    kind="ExternalOutput").ap()
    io["out_lpent"] = nc.dram_tensor("out_lpent", [P, 2, NTILES], F32,
                                     kind="ExternalOutput").ap()
    with tile.TileContext(nc) as tc:
        _emit(tc, io)
    nc.compile()
    _cached["nc"] = nc
    return nc


def _prep_inputs(x, comms, codebook, W0, b0, W1, b1, W2, b2):
    bf = ml_dtypes.bfloat16
    x = np.ascontiguousarray(np.asarray(x, np.float32).reshape(S, H))
    comms = np.ascontiguousarray(np.asarray(comms).reshape(S, NC_, L))
    codebook = np.asarray(codebook, np.float32)
    Ws = [np.asarray(W0, np.float32), np.asarray(W1, np.float32),
          np.asarray(W2, np.float32).copy()]
    bs = np.stack([np.asarray(b, np.float32) for b in (b0, b1, b2)])

    # cumulative-hard transform: cond chunks are [x, S1, S2] with
    # S1 = h0, S2 = h0 + h1, so W2's h0-block absorbs -h1-block.
    Ws[2][:, H:H + 128] -= Ws[2][:, H + 128:H + 256]

    shared = {}
    for l in range(L):
        wt = Ws[l].T.astype(bf)                      # [fan, 2048]
        shared[f"W{l}T"] = np.ascontiguousarray(
            wt.reshape(KC[l], P, FOUT).transpose(1, 0, 2))
    b_hi = bs.astype(bf)
    b_lo = (bs - b_hi.astype(np.float32)).astype(bf)
    shared["biasHL"] = np.ascontiguousarray(np.stack([b_hi, b_lo]))
    shared["ones2"] = np.ones((2, P), bf)
    cb_hi = codebook.astype(bf)
    cbd = np.zeros((P, L, 4, C), bf)
    for l in range(L):
        for ch in range(4):
            cbd[:, l, ch] = cb_hi[l, ch * P:(ch + 1) * P]
    shared["cb"] = cbd
    shared["iotap"] = (np.arange(P)[:, None]
                       + P * np.arange(4)[None, :]).astype(np.float32)
    shared["iotaf"] = np.ascontiguousarray(np.broadcast_to(
        np.arange(FOUT, dtype=np.float16), (P, FOUT)))

    in_maps = []
    for c in range(NCORES):
        tok = slice(c * SC, (c + 1) * SC)
        xc = x[tok]                                   # [2048, 512]
        xT = np.ascontiguousarray(
            xc.T.astype(bf).reshape(4, P, SC).transpose(1, 0, 2))
        cm = comms[tok]                               # [2048, NC, L]
        idxb = np.ascontiguousarray(np.broadcast_to(
            cm.transpose(2, 1, 0).astype(np.float16)[None],
            (P, L, NC_, SC)))
        # idxg[p, l, t*NC+nc] = nc*512 + comms[t*128+p, nc, l]
        cmt = cm.reshape(NTILES, P, NC_, L)
        idxg = (cmt.transpose(1, 3, 0, 2).astype(np.float32)
                + (np.arange(NC_) * V).astype(np.float32)[None, None, None, :])
        idxg = np.ascontiguousarray(idxg.reshape(P, L, NTILES * NC_))
        m = dict(shared)
        m["xT"] = xT
        m["idxb"] = idxb
        m["idxg"] = idxg.astype(np.float32)
        in_maps.append(m)
    return in_maps


def _run(in_maps, **kwargs):
    nc = _build()
    return bass_utils.run_bass_kernel_spmd(
        nc, in_maps, core_ids=list(range(NCORES)), **kwargs)


def _lo_correction(comms, codebook):
    # fp32 - bf16(codebook): the part the on-device bf16 matmul cannot see.
    cb = np.asarray(codebook, np.float32)
    lo = cb - cb.astype(ml_dtypes.bfloat16).astype(np.float32)  # [L, V, C]
    cm = np.asarray(comms).reshape(S, NC_, L)
    out = np.zeros((S, NC_, C), np.float32)
    for l in range(L):
        out += lo[l][cm[:, :, l]]
    return out.reshape(S, NC_ * C)


def _postprocess(results, locorr=None):
    comm = np.concatenate(
        [np.asarray(r["out_hard"], np.float32).T for r in results], axis=0)
    if locorr is not None:
        comm = comm + locorr
    lp = np.concatenate(
        [np.asarray(r["out_lpent"], np.float32)[:, 0].T.reshape(-1)
         for r in results])
    ent = np.concatenate(
        [np.asarray(r["out_lpent"], np.float32)[:, 1].T.reshape(-1)
         for r in results])
    return (comm, lp.reshape(B, T, N), ent.reshape(B, T, N))


def kernel(x, comms, codebook, W0, b0, W1, b1, W2, b2):
    in_maps = _prep_inputs(x, comms, codebook, W0, b0, W1, b1, W2, b2)
    locorr = _lo_correction(comms, codebook)
    res = _run(in_maps)
    return _postprocess(res.results, locorr)
